# revision 4
# baseline (speedup 1.0000x reference)
"""VQ codebook (vector quantization) Trainium2 Bass kernel.

Problem: x [500000, 128] f32, codebook [512, 128] f32.
  xn = l2norm(x); cn = l2norm(codebook)
  distances = |xn|^2 + |cn|^2 - 2 xn@cn.T ; idx = argmin(distances, axis=1)
  quantized = codebook[idx]; losses = mean((quantized - x)^2)
  returns (quantized_ste, codebook_loss, commitment_loss, idx)

Key algebra: argmin_k dist(n,k) == argmax_k xn.cn_k == argmax_k x.cn_k
(positive per-row scale does not change the argmax; the |cn_k|^2 ~= 1
term shifts scores by <1e-6 of the typical top-2 gap). So we compute raw
dots = x @ cn.T with a full-fp32 PE matmul and take a per-row argmax.

Sharding (data parallel over N, 8 cores): each core processes 62592 rows
(489 tiles of 128); core 7's tail is zero-padded. The small codebook is
replicated. Scalar loss partials come back per-partition and are summed
on the host (codebook_loss == commitment_loss in value).

Per 128-row tile on each core:
  DMA x tile -> PE transpose -> ACT copy to SBUF -> fp32 matmul against
  cnT [128,512] -> single-pass DVE scan-argmax (custom op: running-max
  scan, record-breaker positions, max-accumulated index) -> int cast ->
  indirect-DMA gather of codebook rows -> fused (q-x)^2 accumulate ->
  DMA q tile out.
"""

import sys

if "/opt/trn_rl_repo" not in sys.path:
    sys.path.insert(0, "/opt/trn_rl_repo")

import numpy as np

N, D, K = 500000, 128, 512
N_CORES = 8
TILES = 489
ROWS_PC = TILES * 128  # 62592 rows per core; 8*62592 = 500736 >= N

_CACHE = {}


def _register_dve_ops():
    """Define + register the two custom single-pass DVE ops at runtime."""
    import concourse.dve_ops as dve_ops
    from concourse.dve_ops import DveOp
    from concourse.dve_spec import (
        AluOp,
        C0,
        Idx,
        MaxNeg,
        Spec,
        Src0,
        Src1,
        eq,
        lower,
        scan,
        select,
        sq,
    )
    from concourse.dve_uop import DveOpSpec

    if "ARGMAX_SCAN_ANT" in dve_ops._SUB_OPCODE_FOR_NAME:
        by_name = {op.name: op for op in dve_ops.OPS}
        return by_name["ARGMAX_SCAN_ANT"], by_name["SQDIFF_ACC_ANT"]

    def _argmax_ref(in0):
        run = np.maximum.accumulate(in0, axis=-1)
        idxs = np.arange(in0.shape[-1], dtype=np.float32)
        out = np.where(in0 == run, idxs, np.float32(-3.4028235e38))
        acc = out.max(axis=-1, keepdims=True)
        return out.astype(np.float32), acc.astype(np.float32)

    def _sqdiff_ref(in0, in1, s0):
        out = (in0 - in1) ** 2
        acc = s0 + out.reshape(out.shape[0], -1).sum(-1, keepdims=True)
        return out.astype(np.float32), acc.astype(np.float32)

    r = scan(AluOp.MAX, Src0)
    argmax_spec = Spec(
        body=select(eq(Src0, r), Idx, MaxNeg),
        accum=AluOp.MAX,
        reference=_argmax_ref,
    )
    sqdiff_spec = Spec(
        body=sq(Src0 - Src1),
        accum=AluOp.ADD,
        accum_init=C0,
        reference=_sqdiff_ref,
    )

    ops = []
    for name, spec, rd1 in (
        ("ARGMAX_SCAN_ANT", argmax_spec, False),
        ("SQDIFF_ACC_ANT", sqdiff_spec, True),
    ):
        shas = {}
        for ver in ("v3", "v4"):
            uops = lower(spec, ver=ver)
            shas[ver] = DveOpSpec(name=name, opcode=0, uops=uops, rd1_en=rd1).sha(ver)
        op = DveOp(name, spec, subdim=False, uops_sha=shas)
        dve_ops._SUB_OPCODE_FOR_NAME[name] = dve_ops._CUSTOM_DVE_ROW_BASE + len(
            dve_ops.OPS
        )
        dve_ops.OPS.append(op)
        ops.append(op)
    return ops[0], ops[1]


def _build():
    import concourse.bass as bass
    import concourse.tile as tile
    from concourse import bacc, masks, mybir

    ARGMAX, SQDIFF = _register_dve_ops()
    f32 = mybir.dt.float32

    nc = bacc.Bacc("TRN2", target_bir_lowering=False, debug=False,
                   num_devices=N_CORES)
    x_ap = nc.dram_tensor("x", [ROWS_PC, D], f32, kind="ExternalInput").ap()
    cb_ap = nc.dram_tensor("codebook", [K, D], f32, kind="ExternalInput").ap()
    q_ap = nc.dram_tensor("q", [ROWS_PC, D], f32, kind="ExternalOutput").ap()
    idx_ap = nc.dram_tensor("idx", [ROWS_PC], mybir.dt.int32,
                            kind="ExternalOutput").ap()
    loss_ap = nc.dram_tensor("loss", [128, 1], f32, kind="ExternalOutput").ap()

    with tile.TileContext(nc) as tc:
        with (
            tc.tile_pool(name="singles", bufs=1) as singles,
            tc.tile_pool(name="xp", bufs=6) as x_pool,
            tc.tile_pool(name="xtp", bufs=3) as xt_pool,
            tc.tile_pool(name="qp", bufs=4) as q_pool,
            tc.tile_pool(name="idxfp", bufs=3) as idxf_pool,
            tc.tile_pool(name="psA", bufs=2, space="PSUM") as psA,
            tc.tile_pool(name="psB", bufs=3, space="PSUM") as psB,
        ):
            # ---------------- preamble: identity, codebook prep -------------
            ident = singles.tile([128, 128], f32, tag="ident")
            masks.make_identity(nc, ident[:])

            cb_sb = singles.tile([128, 4, D], f32, tag="cb_sb")
            nc.sync.dma_start(cb_sb[:], cb_ap.rearrange("(c p) d -> p c d", p=128))

            cn_sb = singles.tile([128, 4, D], f32, tag="cn_sb")
            cnT = singles.tile([128, K], f32, tag="cnT")
            sq_junk = singles.tile([128, D], f32, tag="sq_junk")
            s2 = singles.tile([128, 4], f32, tag="s2")
            for c in range(4):
                nc.scalar.activation(
                    sq_junk[:], cb_sb[:, c, :],
                    mybir.ActivationFunctionType.Square,
                    accum_out=s2[:, c : c + 1],
                )
            snorm = singles.tile([128, 4], f32, tag="snorm")
            nc.scalar.sqrt(snorm[:], s2[:])
            nc.vector.tensor_scalar_max(snorm[:], snorm[:], 1e-12)
            sinv = singles.tile([128, 4], f32, tag="sinv")
            nc.vector.reciprocal(sinv[:], snorm[:])
            for c in range(4):
                nc.vector.tensor_scalar_mul(
                    cn_sb[:, c, :], cb_sb[:, c, :], sinv[:, c : c + 1]
                )
            for c in range(4):
                tp = psA.tile([128, 128], f32, tag="tp")
                nc.tensor.transpose(tp[:], cn_sb[:, c, :], ident[:])
                nc.scalar.copy(cnT[:, c * 128 : (c + 1) * 128], tp[:])

            loss_acc = singles.tile([128, 1], f32, tag="loss_acc")
            nc.vector.memset(loss_acc[:], 0.0)
            idx_all = singles.tile([128, TILES], mybir.dt.int32, tag="idx_all")
            junk = singles.tile([128, K], f32, tag="junk")
            junk2 = singles.tile([128, D], f32, tag="junk2")

            # ---------------- main loop -------------------------------------
            xs = {}
            xTs = {}

            def load_and_transpose(t):
                xs[t] = x_pool.tile([128, D], f32, tag="x_t", name=f"x_t{t}")
                nc.sync.dma_start(xs[t][:], x_ap[t * 128 : (t + 1) * 128, :])
                xTs[t] = psA.tile([128, 128], f32, tag="tp", name=f"xT_ps{t}")
                nc.tensor.transpose(xTs[t][:], xs[t][:], ident[:])

            load_and_transpose(0)
            for t in range(TILES):
                if t + 1 < TILES:
                    load_and_transpose(t + 1)
                xT_sb = xt_pool.tile([128, 128], f32, tag="xT_sb")
                nc.scalar.copy(xT_sb[:], xTs[t][:])
                dots = psB.tile([128, K], f32, tag="dots")
                nc.tensor.matmul(dots[:], xT_sb[:], cnT[:], start=True, stop=True)

                idxf = idxf_pool.tile([128, 1], f32, tag="idxf")
                nc.vector._custom_dve(
                    ARGMAX, out=junk[:], accum_out=idxf[:], in0=dots[:]
                )
                nc.vector.tensor_copy(idx_all[:, t : t + 1], idxf[:])

                q_sb = q_pool.tile([128, D], f32, tag="q_t")
                nc.gpsimd.indirect_dma_start(
                    out=q_sb[:],
                    out_offset=None,
                    in_=cb_ap,
                    in_offset=bass.IndirectOffsetOnAxis(
                        ap=idx_all[:, t : t + 1], axis=0
                    ),
                )
                nc.vector._custom_dve(
                    SQDIFF,
                    out=junk2[:],
                    accum_out=loss_acc[:],
                    in0=q_sb[:],
                    in1=xs[t][:],
                    s0=loss_acc[:],
                )
                nc.scalar.dma_start(q_ap[t * 128 : (t + 1) * 128, :], q_sb[:])
                del xs[t], xTs[t]

            # ---------------- epilogue --------------------------------------
            nc.sync.dma_start(idx_ap.rearrange("(t p) -> p t", p=128), idx_all[:])
            nc.sync.dma_start(loss_ap, loss_acc[:])

    nc.compile()
    return nc


def _get_compiled():
    if "nc" not in _CACHE:
        _CACHE["nc"] = _build()
    return _CACHE["nc"]


def kernel(x, codebook, _want_results=False, _trace=False):
    from concourse.bass_utils import run_bass_kernel_spmd

    x = np.ascontiguousarray(np.asarray(x, dtype=np.float32))
    cb = np.ascontiguousarray(np.asarray(codebook, dtype=np.float32))
    assert x.shape == (N, D) and cb.shape == (K, D)

    nc = _get_compiled()
    in_maps = []
    for c in range(N_CORES):
        lo, hi = c * ROWS_PC, (c + 1) * ROWS_PC
        if hi <= N:
            xs = x[lo:hi]
        else:
            xs = np.zeros((ROWS_PC, D), np.float32)
            xs[: N - lo] = x[lo:N]
        in_maps.append({"x": xs, "codebook": cb})

    res = run_bass_kernel_spmd(
        nc, in_maps, core_ids=list(range(N_CORES)), trace=_trace
    )
    q = np.concatenate([r["q"] for r in res.results], axis=0)[:N]
    idx = np.concatenate([r["idx"] for r in res.results], axis=0)[:N].astype(
        np.int32
    )
    total = 0.0
    for r in res.results:
        total += float(r["loss"].astype(np.float64).sum())
    loss = np.float32(total / (N * D))
    out = (q, loss, loss, idx)
    if _want_results:
        return out, res
    return out


# revision 6
# speedup vs baseline: 1.2545x; 1.2545x over previous
"""VQ codebook (vector quantization) Trainium2 Bass kernel.

Problem: x [500000, 128] f32, codebook [512, 128] f32.
  xn = l2norm(x); cn = l2norm(codebook)
  distances = |xn|^2 + |cn|^2 - 2 xn@cn.T ; idx = argmin(distances, axis=1)
  quantized = codebook[idx]; losses = mean((quantized - x)^2)
  returns (quantized_ste, codebook_loss, commitment_loss, idx)

Key algebra: argmin_k dist(n,k) == argmax_k xn.cn_k == argmax_k x.cn_k
(positive per-row scale does not change the argmax; the |cn_k|^2 ~= 1
term shifts scores by <1e-6 of the typical top-2 gap). So we compute raw
dots = x @ cn.T with a full-fp32 PE matmul and take a per-row argmax.

Sharding (data parallel over N, 8 cores): each core processes 62592 rows
(489 tiles of 128); core 7's tail is zero-padded. The small codebook is
replicated. Scalar loss partials come back per-partition and are summed
on the host (codebook_loss == commitment_loss in value).

x is uploaded TRANSPOSED (xT [128, rows]) so each row-tile is directly a
matmul stationary operand - no per-tile PE transpose / PSUM round trip.
Per 128-row tile: fp32 matmul against cnT [128,512] -> single-pass DVE
scan-argmax (custom op: running-max scan, record-breaker positions,
max-accumulated index). Per group of 3 tiles: batched int cast, batched
indirect-DMA gather of codebook rows, PE transposes of q for the fused
(qT-xT)^2 loss accumulate, batched q DMA out.
"""

import sys

if "/opt/trn_rl_repo" not in sys.path:
    sys.path.insert(0, "/opt/trn_rl_repo")

import numpy as np

N, D, K = 500000, 128, 512
N_CORES = 8
G = 3
NGRP = 163
TILES = G * NGRP  # 489
ROWS_PC = TILES * 128  # 62592 rows per core; 8*62592 = 500736 >= N

_CACHE = {}


def _register_dve_ops():
    """Define + register the two custom single-pass DVE ops at runtime."""
    import concourse.dve_ops as dve_ops
    from concourse.dve_ops import DveOp
    from concourse.dve_spec import (
        AluOp,
        C0,
        Idx,
        MaxNeg,
        Spec,
        Src0,
        Src1,
        eq,
        lower,
        scan,
        select,
        sq,
    )
    from concourse.dve_uop import DveOpSpec

    if "ARGMAX_SCAN_ANT" in dve_ops._SUB_OPCODE_FOR_NAME:
        by_name = {op.name: op for op in dve_ops.OPS}
        return by_name["ARGMAX_SCAN_ANT"], by_name["SQDIFF_ACC_ANT"]

    def _argmax_ref(in0):
        run = np.maximum.accumulate(in0, axis=-1)
        idxs = np.arange(in0.shape[-1], dtype=np.float32)
        out = np.where(in0 == run, idxs, np.float32(-3.4028235e38))
        acc = out.max(axis=-1, keepdims=True)
        return out.astype(np.float32), acc.astype(np.float32)

    def _sqdiff_ref(in0, in1, s0):
        out = (in0 - in1) ** 2
        acc = s0 + out.reshape(out.shape[0], -1).sum(-1, keepdims=True)
        return out.astype(np.float32), acc.astype(np.float32)

    r = scan(AluOp.MAX, Src0)
    argmax_spec = Spec(
        body=select(eq(Src0, r), Idx, MaxNeg),
        accum=AluOp.MAX,
        reference=_argmax_ref,
    )
    sqdiff_spec = Spec(
        body=sq(Src0 - Src1),
        accum=AluOp.ADD,
        accum_init=C0,
        reference=_sqdiff_ref,
    )

    ops = []
    for name, spec, rd1 in (
        ("ARGMAX_SCAN_ANT", argmax_spec, False),
        ("SQDIFF_ACC_ANT", sqdiff_spec, True),
    ):
        shas = {}
        for ver in ("v3", "v4"):
            uops = lower(spec, ver=ver)
            shas[ver] = DveOpSpec(name=name, opcode=0, uops=uops, rd1_en=rd1).sha(ver)
        op = DveOp(name, spec, subdim=False, uops_sha=shas)
        dve_ops._SUB_OPCODE_FOR_NAME[name] = dve_ops._CUSTOM_DVE_ROW_BASE + len(
            dve_ops.OPS
        )
        dve_ops.OPS.append(op)
        ops.append(op)
    return ops[0], ops[1]


def _build():
    import concourse.bass as bass
    import concourse.tile as tile
    from concourse import bacc, masks, mybir

    ARGMAX, SQDIFF = _register_dve_ops()
    f32 = mybir.dt.float32
    GW = G * 128  # group width in rows

    nc = bacc.Bacc("TRN2", target_bir_lowering=False, debug=False,
                   num_devices=N_CORES)
    xT_ap = nc.dram_tensor("xT", [D, ROWS_PC], f32, kind="ExternalInput").ap()
    cb_ap = nc.dram_tensor("codebook", [K, D], f32, kind="ExternalInput").ap()
    q_ap = nc.dram_tensor("q", [ROWS_PC, D], f32, kind="ExternalOutput").ap()
    idx_ap = nc.dram_tensor("idx", [ROWS_PC], mybir.dt.int32,
                            kind="ExternalOutput").ap()
    loss_ap = nc.dram_tensor("loss", [128, 1], f32, kind="ExternalOutput").ap()

    with tile.TileContext(nc) as tc:
        with (
            tc.tile_pool(name="singles", bufs=1) as singles,
            tc.tile_pool(name="xtp", bufs=5) as xt_pool,
            tc.tile_pool(name="qp", bufs=3) as q_pool,
            tc.tile_pool(name="idxfp", bufs=3) as idxf_pool,
            tc.tile_pool(name="psB", bufs=4, space="PSUM") as psB,
            tc.tile_pool(name="psC", bufs=2, space="PSUM") as psC,
            tc.tile_pool(name="psPre", bufs=1, space="PSUM") as psPre,
        ):
            # ---------------- preamble: identity, codebook prep -------------
            ident = singles.tile([128, 128], f32, tag="ident")
            masks.make_identity(nc, ident[:])

            cb_sb = singles.tile([128, 4, D], f32, tag="cb_sb")
            nc.sync.dma_start(cb_sb[:], cb_ap.rearrange("(c p) d -> p c d", p=128))

            cn_sb = singles.tile([128, 4, D], f32, tag="cn_sb")
            cnT = singles.tile([128, K], f32, tag="cnT")
            sq_junk = singles.tile([128, D], f32, tag="sq_junk")
            s2 = singles.tile([128, 4], f32, tag="s2")
            for c in range(4):
                nc.scalar.activation(
                    sq_junk[:], cb_sb[:, c, :],
                    mybir.ActivationFunctionType.Square,
                    accum_out=s2[:, c : c + 1],
                )
            snorm = singles.tile([128, 4], f32, tag="snorm")
            nc.scalar.sqrt(snorm[:], s2[:])
            nc.vector.tensor_scalar_max(snorm[:], snorm[:], 1e-12)
            sinv = singles.tile([128, 4], f32, tag="sinv")
            nc.vector.reciprocal(sinv[:], snorm[:])
            for c in range(4):
                nc.vector.tensor_scalar_mul(
                    cn_sb[:, c, :], cb_sb[:, c, :], sinv[:, c : c + 1]
                )
            for c in range(4):
                tp = psPre.tile([128, 128], f32, tag="tp", name=f"cn_tp{c}")
                nc.tensor.transpose(tp[:], cn_sb[:, c, :], ident[:])
                nc.scalar.copy(cnT[:, c * 128 : (c + 1) * 128], tp[:])

            loss_acc = singles.tile([128, 1], f32, tag="loss_acc")
            nc.vector.memset(loss_acc[:], 0.0)
            idx_all = singles.tile([128, TILES], mybir.dt.int32, tag="idx_all")
            junk = singles.tile([128, K], f32, tag="junk")
            junk2 = singles.tile([128, GW], f32, tag="junk2")

            # ---------------- main loop (groups of G row-tiles) -------------
            for g in range(NGRP):
                xT_sb = xt_pool.tile([128, GW], f32, tag="xT_sb",
                                     name=f"xT{g}")
                nc.sync.dma_start(xT_sb[:], xT_ap[:, g * GW : (g + 1) * GW])

                idxf = idxf_pool.tile([128, G], f32, tag="idxf",
                                      name=f"idxf{g}")
                for j in range(G):
                    dots = psB.tile([128, K], f32, tag="dots",
                                    name=f"dots{g}_{j}")
                    nc.tensor.matmul(
                        dots[:], xT_sb[:, j * 128 : (j + 1) * 128], cnT[:],
                        start=True, stop=True,
                    )
                    nc.vector._custom_dve(
                        ARGMAX, out=junk[:], accum_out=idxf[:, j : j + 1],
                        in0=dots[:],
                    )
                nc.vector.tensor_copy(idx_all[:, g * G : (g + 1) * G], idxf[:])

                q_sb = q_pool.tile([128, G, D], f32, tag="q_sb", name=f"q{g}")
                for j in range(G):
                    t = g * G + j
                    nc.gpsimd.indirect_dma_start(
                        out=q_sb[:, j, :],
                        out_offset=None,
                        in_=cb_ap,
                        in_offset=bass.IndirectOffsetOnAxis(
                            ap=idx_all[:, t : t + 1], axis=0
                        ),
                    )
                qT = psC.tile([128, GW], f32, tag="qT", name=f"qT{g}")
                for j in range(G):
                    nc.tensor.transpose(
                        qT[:, j * 128 : (j + 1) * 128], q_sb[:, j, :], ident[:]
                    )
                nc.vector._custom_dve(
                    SQDIFF,
                    out=junk2[:],
                    accum_out=loss_acc[:],
                    in0=qT[:],
                    in1=xT_sb[:],
                    s0=loss_acc[:],
                )
                nc.scalar.dma_start(
                    q_ap[g * GW : (g + 1) * GW, :].rearrange(
                        "(j p) d -> p j d", p=128
                    ),
                    q_sb[:],
                )

            # ---------------- epilogue --------------------------------------
            nc.sync.dma_start(idx_ap.rearrange("(t p) -> p t", p=128), idx_all[:])
            nc.sync.dma_start(loss_ap, loss_acc[:])

    nc.compile()
    return nc


def _get_compiled():
    if "nc" not in _CACHE:
        _CACHE["nc"] = _build()
    return _CACHE["nc"]


def kernel(x, codebook, _want_results=False, _trace=False):
    from concourse.bass_utils import run_bass_kernel_spmd

    x = np.asarray(x, dtype=np.float32)
    cb = np.ascontiguousarray(np.asarray(codebook, dtype=np.float32))
    assert x.shape == (N, D) and cb.shape == (K, D)

    nc = _get_compiled()
    in_maps = []
    for c in range(N_CORES):
        lo, hi = c * ROWS_PC, (c + 1) * ROWS_PC
        if hi <= N:
            xT = np.ascontiguousarray(x[lo:hi].T)
        else:
            xT = np.zeros((D, ROWS_PC), np.float32)
            xT[:, : N - lo] = x[lo:N].T
        in_maps.append({"xT": xT, "codebook": cb})

    res = run_bass_kernel_spmd(
        nc, in_maps, core_ids=list(range(N_CORES)), trace=_trace
    )
    q = np.concatenate([r["q"] for r in res.results], axis=0)[:N]
    idx = np.concatenate([r["idx"] for r in res.results], axis=0)[:N].astype(
        np.int32
    )
    total = 0.0
    for r in res.results:
        total += float(r["loss"].astype(np.float64).sum())
    loss = np.float32(total / (N * D))
    out = (q, loss, loss, idx)
    if _want_results:
        return out, res
    return out


# revision 9
# speedup vs baseline: 1.5765x; 1.2567x over previous
"""VQ codebook (vector quantization) Trainium2 Bass kernel.

Problem: x [500000, 128] f32, codebook [512, 128] f32.
  xn = l2norm(x); cn = l2norm(codebook)
  distances = |xn|^2 + |cn|^2 - 2 xn@cn.T ; idx = argmin(distances, axis=1)
  quantized = codebook[idx]; losses = mean((quantized - x)^2)
  returns (quantized_ste, codebook_loss, commitment_loss, idx)

Key algebra: argmin_k dist(n,k) == argmax_k xn.cn_k == argmax_k x.cn_k
(positive per-row scale does not change the argmax; the |cn_k|^2 ~= 1
term shifts scores by <1e-6 of the typical top-2 gap). So we compute raw
dots = x @ cn.T with a full-fp32 PE matmul and take a per-row argmax.

Sharding (data parallel over N, 8 cores): each core processes 62592 rows
(489 tiles of 128); core 7's tail is zero-padded. The small codebook is
replicated. Scalar loss partials come back per-partition and are summed
on the host (codebook_loss == commitment_loss in value).

x is uploaded TRANSPOSED (xT [128, rows]) so each row-tile is directly a
matmul stationary operand - no per-tile PE transpose / PSUM round trip.
Per 128-row tile: fp32 matmul against cnT [128,512] -> single-pass DVE
scan-argmax (custom op: running-max scan, record-breaker positions,
max-accumulated index). Per group of 3 tiles: batched int cast, batched
indirect-DMA gather of codebook rows, PE transposes of q for the fused
(qT-xT)^2 loss accumulate, batched q DMA out.
"""

import sys

if "/opt/trn_rl_repo" not in sys.path:
    sys.path.insert(0, "/opt/trn_rl_repo")

import numpy as np

N, D, K = 500000, 128, 512
N_CORES = 8
G = 3
NGRP = 163
TILES = G * NGRP  # 489
ROWS_PC = TILES * 128  # 62592 rows per core; 8*62592 = 500736 >= N

_CACHE = {}


def _register_dve_ops():
    """Define + register the two custom single-pass DVE ops at runtime."""
    import concourse.dve_ops as dve_ops
    from concourse.dve_ops import DveOp
    from concourse.dve_spec import (
        AluOp,
        C0,
        Idx,
        MaxNeg,
        Spec,
        Src0,
        Src1,
        eq,
        lower,
        scan,
        select,
        sq,
    )
    from concourse.dve_uop import DveOpSpec

    if "ARGMAX_SCAN_ANT" in dve_ops._SUB_OPCODE_FOR_NAME:
        by_name = {op.name: op for op in dve_ops.OPS}
        return by_name["ARGMAX_SCAN_ANT"], by_name["SQDIFF_ACC_ANT"]

    def _argmax_ref(in0):
        run = np.maximum.accumulate(in0, axis=-1)
        idxs = np.arange(in0.shape[-1], dtype=np.float32)
        out = np.where(in0 == run, idxs, np.float32(-3.4028235e38))
        acc = out.max(axis=-1, keepdims=True)
        return out.astype(np.float32), acc.astype(np.float32)

    def _sqdiff_ref(in0, in1, s0):
        out = (in0 - in1) ** 2
        acc = s0 + out.reshape(out.shape[0], -1).sum(-1, keepdims=True)
        return out.astype(np.float32), acc.astype(np.float32)

    r = scan(AluOp.MAX, Src0)
    argmax_spec = Spec(
        body=select(eq(Src0, r), Idx, MaxNeg),
        accum=AluOp.MAX,
        reference=_argmax_ref,
    )
    sqdiff_spec = Spec(
        body=sq(Src0 - Src1),
        accum=AluOp.ADD,
        accum_init=C0,
        reference=_sqdiff_ref,
    )

    ops = []
    for name, spec, rd1 in (
        ("ARGMAX_SCAN_ANT", argmax_spec, False),
        ("SQDIFF_ACC_ANT", sqdiff_spec, True),
    ):
        shas = {}
        for ver in ("v3", "v4"):
            uops = lower(spec, ver=ver)
            shas[ver] = DveOpSpec(name=name, opcode=0, uops=uops, rd1_en=rd1).sha(ver)
        op = DveOp(name, spec, subdim=False, uops_sha=shas)
        dve_ops._SUB_OPCODE_FOR_NAME[name] = dve_ops._CUSTOM_DVE_ROW_BASE + len(
            dve_ops.OPS
        )
        dve_ops.OPS.append(op)
        ops.append(op)
    return ops[0], ops[1]


def _build():
    import concourse.bass as bass
    import concourse.tile as tile
    from concourse import bacc, masks, mybir

    ARGMAX, SQDIFF = _register_dve_ops()
    f32 = mybir.dt.float32
    GW = G * 128  # group width in rows

    nc = bacc.Bacc("TRN2", target_bir_lowering=False, debug=False,
                   num_devices=N_CORES)
    xT_ap = nc.dram_tensor("xT", [D, ROWS_PC], f32, kind="ExternalInput").ap()
    cb_ap = nc.dram_tensor("codebook", [K, D], f32, kind="ExternalInput").ap()
    q_ap = nc.dram_tensor("q", [ROWS_PC, D], f32, kind="ExternalOutput").ap()
    # idx laid out [128, TILES] partition-major; host reorders (idx.T.ravel())
    idx_ap = nc.dram_tensor("idx", [128, TILES], mybir.dt.int32,
                            kind="ExternalOutput").ap()
    loss_ap = nc.dram_tensor("loss", [128, 1], f32, kind="ExternalOutput").ap()

    with tile.TileContext(nc) as tc:
        with (
            tc.tile_pool(name="singles", bufs=1) as singles,
            tc.tile_pool(name="xtp", bufs=6) as xt_pool,
            tc.tile_pool(name="qp", bufs=5) as q_pool,
            tc.tile_pool(name="idxfp", bufs=3) as idxf_pool,
            tc.tile_pool(name="psB", bufs=4, space="PSUM") as psB,
            tc.tile_pool(name="psC", bufs=2, space="PSUM") as psC,
            tc.tile_pool(name="psPre", bufs=1, space="PSUM") as psPre,
        ):
            # ---------------- preamble: identity, codebook prep -------------
            ident = singles.tile([128, 128], f32, tag="ident")
            masks.make_identity(nc, ident[:])

            cb_sb = singles.tile([128, 4, D], f32, tag="cb_sb")
            nc.sync.dma_start(cb_sb[:], cb_ap.rearrange("(c p) d -> p c d", p=128))

            cn_sb = singles.tile([128, 4, D], f32, tag="cn_sb")
            cnT = singles.tile([128, K], f32, tag="cnT")
            sq_junk = singles.tile([128, D], f32, tag="sq_junk")
            s2 = singles.tile([128, 4], f32, tag="s2")
            for c in range(4):
                nc.scalar.activation(
                    sq_junk[:], cb_sb[:, c, :],
                    mybir.ActivationFunctionType.Square,
                    accum_out=s2[:, c : c + 1],
                )
            snorm = singles.tile([128, 4], f32, tag="snorm")
            nc.scalar.sqrt(snorm[:], s2[:])
            nc.vector.tensor_scalar_max(snorm[:], snorm[:], 1e-12)
            sinv = singles.tile([128, 4], f32, tag="sinv")
            nc.vector.reciprocal(sinv[:], snorm[:])
            for c in range(4):
                nc.vector.tensor_scalar_mul(
                    cn_sb[:, c, :], cb_sb[:, c, :], sinv[:, c : c + 1]
                )
            for c in range(4):
                tp = psPre.tile([128, 128], f32, tag="tp", name=f"cn_tp{c}")
                nc.tensor.transpose(tp[:], cn_sb[:, c, :], ident[:])
                nc.scalar.copy(cnT[:, c * 128 : (c + 1) * 128], tp[:])

            loss_acc = singles.tile([128, 1], f32, tag="loss_acc")
            nc.vector.memset(loss_acc[:], 0.0)
            idx_all = singles.tile([128, TILES], mybir.dt.int32, tag="idx_all")
            junk = singles.tile([128, K], f32, tag="junk")
            junk2 = singles.tile([128, GW], f32, tag="junk2")

            # ---------------- main loop (groups of G row-tiles) -------------
            # Loss (q transposes + sqdiff) for group g is deferred DELAY
            # groups so the PE's qT transposes never wait on an in-flight
            # gather (keeps the PE dense -> HAM stays at full clock).
            DELAY = 2
            xTs = {}
            qs = {}

            def emit_loss(h):
                qT = psC.tile([128, GW], f32, tag="qT", name=f"qT{h}")
                for j in range(G):
                    nc.tensor.transpose(
                        qT[:, j * 128 : (j + 1) * 128], qs[h][:, j, :], ident[:]
                    )
                nc.vector._custom_dve(
                    SQDIFF,
                    out=junk2[:],
                    accum_out=loss_acc[:],
                    in0=qT[:],
                    in1=xTs[h][:],
                    s0=loss_acc[:],
                )
                del qs[h], xTs[h]

            for g in range(NGRP):
                xT_sb = xt_pool.tile([128, GW], f32, tag="xT_sb",
                                     name=f"xT{g}")
                xTs[g] = xT_sb
                nc.sync.dma_start(xT_sb[:], xT_ap[:, g * GW : (g + 1) * GW])

                idxf = idxf_pool.tile([128, G], f32, tag="idxf",
                                      name=f"idxf{g}")
                for j in range(G):
                    dots = psB.tile([128, K], f32, tag="dots",
                                    name=f"dots{g}_{j}")
                    nc.tensor.matmul(
                        dots[:], xT_sb[:, j * 128 : (j + 1) * 128], cnT[:],
                        start=True, stop=True,
                    )
                    nc.vector._custom_dve(
                        ARGMAX, out=junk[:], accum_out=idxf[:, j : j + 1],
                        in0=dots[:],
                    )
                nc.vector.tensor_copy(idx_all[:, g * G : (g + 1) * G], idxf[:])

                q_sb = q_pool.tile([128, G, D], f32, tag="q_sb", name=f"q{g}")
                qs[g] = q_sb
                for j in range(G):
                    t = g * G + j
                    nc.gpsimd.indirect_dma_start(
                        out=q_sb[:, j, :],
                        out_offset=None,
                        in_=cb_ap,
                        in_offset=bass.IndirectOffsetOnAxis(
                            ap=idx_all[:, t : t + 1], axis=0
                        ),
                    )
                nc.scalar.dma_start(
                    q_ap[g * GW : (g + 1) * GW, :].rearrange(
                        "(j p) d -> p j d", p=128
                    ),
                    q_sb[:],
                )
                if g >= DELAY:
                    emit_loss(g - DELAY)

            for h in range(NGRP - DELAY, NGRP):
                emit_loss(h)

            # ---------------- epilogue --------------------------------------
            nc.sync.dma_start(idx_ap, idx_all[:])
            nc.sync.dma_start(loss_ap, loss_acc[:])

    nc.compile()
    return nc


def _get_compiled():
    if "nc" not in _CACHE:
        _CACHE["nc"] = _build()
    return _CACHE["nc"]


def kernel(x, codebook, _want_results=False, _trace=False):
    from concourse.bass_utils import run_bass_kernel_spmd

    x = np.asarray(x, dtype=np.float32)
    cb = np.ascontiguousarray(np.asarray(codebook, dtype=np.float32))
    assert x.shape == (N, D) and cb.shape == (K, D)

    nc = _get_compiled()
    in_maps = []
    for c in range(N_CORES):
        lo, hi = c * ROWS_PC, (c + 1) * ROWS_PC
        if hi <= N:
            xT = np.ascontiguousarray(x[lo:hi].T)
        else:
            xT = np.zeros((D, ROWS_PC), np.float32)
            xT[:, : N - lo] = x[lo:N].T
        in_maps.append({"xT": xT, "codebook": cb})

    res = run_bass_kernel_spmd(
        nc, in_maps, core_ids=list(range(N_CORES)), trace=_trace
    )
    q = np.concatenate([r["q"] for r in res.results], axis=0)[:N]
    # device idx layout is [128, TILES] partition-major; row t*128+p = [p, t]
    idx = np.concatenate(
        [r["idx"].T.reshape(-1) for r in res.results], axis=0
    )[:N].astype(np.int32)
    total = 0.0
    for r in res.results:
        total += float(r["loss"].astype(np.float64).sum())
    loss = np.float32(total / (N * D))
    out = (q, loss, loss, idx)
    if _want_results:
        return out, res
    return out
